# revision 7
# baseline (speedup 1.0000x reference)
"""Trainium2 Bass kernel for CrossModalAttention.

Full (unsharded) inputs in, full output out. Internally: data-parallel over
batch across 8 NeuronCores (B=16 -> 2 batches per core), one SPMD Bass/Tile
program per core, executed via run_bass_kernel_spmd.

Per-core algorithm (per batch):
  1. PE-transpose Wk/Wv (and Wq once) into [d, h] layout in SBUF.
  2. KV phase: stream kv in 512-row blocks; PE-transpose to kv^T; project to
     K^T [768, 2048] (fp32, +bias via ACT) and V [2048, 768] (bf16, +bias via
     DVE), both SBUF-resident.
  3. Attention phase: stream xq in 512-row blocks; PE-transpose; project to
     Q^T; per 128-row q-tile: S = Q^T.T @ K^T in two 1024-wide PSUM halves
     (float32r matmuls), softmax via ACT Exp with fused row-sum (max offset
     taken from the first half only - safe, softmax is shift-invariant),
     P stored bf16, PE-transposed to P^T, then O = P^T.T @ V accumulated over
     16 k-chunks, scaled by 1/sum on the way out.

Matmul dtype: float32r (TF32-like, full PE rate at moving-dim >= 256) for the
projections and scores; bf16 for P/V in the second attention matmul.
"""

import numpy as np
from contextlib import ExitStack

import concourse.bass as bass
import concourse.mybir as mybir
import concourse.tile as tile
from concourse import bacc
from concourse.bass_utils import run_bass_kernel_spmd
from concourse.masks import make_identity

F32 = mybir.dt.float32
F32R = mybir.dt.float32r
BF16 = mybir.dt.bfloat16
AX_X = mybir.AxisListType.X
AF = mybir.ActivationFunctionType

B, QLEN, KVLEN = 16, 2048, 2048
DQ, DKV, H = 768, 1024, 768
NCORES = 8
BPC = B // NCORES  # batches per core
P = 128
NH = H // P    # 6 h-chunks
NDQ = DQ // P  # 6 d-chunks (query dim)
NDK = DKV // P # 8 d-chunks (kv dim)
BLK = 512
KB = KVLEN // BLK  # 4 kv blocks
QB = QLEN // BLK   # 4 q blocks
NKT = KVLEN // P   # 16 kv tiles of 128


def _transpose_weight(tc, nat_pool, psum_pool, w_ap, wt_tile, nd, ident, nm):
    """w_ap: DRAM [H, nd*128] (torch Linear weight layout [out, in]).
    wt_tile: SBUF f32r [128, nd, H] holding W^T, i.e. wt[p, d, h] = W[h, d*128+p]."""
    nc = tc.nc
    nats = []
    for h in range(NH):
        wn = nat_pool.tile([P, nd * P], F32, name=f"wn_{nm}_{h}", tag=f"wn{h}")
        nc.sync.dma_start(out=wn, in_=w_ap[h * P:(h + 1) * P, :])
        nats.append(wn)
    for d in range(nd):
        ps = psum_pool.tile([P, H], F32, name=f"wtp_{nm}_{d}", tag="wtp")
        for h in range(NH):
            nc.tensor.transpose(ps[:, h * P:(h + 1) * P],
                                nats[h][:, d * P:(d + 1) * P], ident)
        nc.vector.tensor_copy(out=wt_tile[:, d, :], in_=ps)


def _emit(tc, xq, kvm, wq, bq, wk, bk, wv, bv, out):
    nc = tc.nc
    with ExitStack() as ctx:
        singles = ctx.enter_context(tc.tile_pool(name="singles", bufs=1))
        ident = singles.tile([P, P], F32, name="ident")
        make_identity(nc, ident)
        identb = singles.tile([P, P], BF16, name="identb")
        make_identity(nc, identb)
        # biases: bq/bk as [128, 6] (per-partition scalars per h-chunk),
        # bv broadcast to all partitions (added along the free dim of V)
        bqs = singles.tile([P, NH], F32, name="bqs")
        nc.gpsimd.dma_start(out=bqs, in_=bq.rearrange("(t p) -> p t", p=P))
        bks = singles.tile([P, NH], F32, name="bks")
        nc.gpsimd.dma_start(out=bks, in_=bk.rearrange("(t p) -> p t", p=P))
        bvb = singles.tile([P, H], F32, name="bvb")
        bv_bcast = bass.AP(tensor=bv.tensor, offset=bv.offset,
                           ap=[[0, P]] + list(bv.ap))
        nc.gpsimd.dma_start(out=bvb, in_=bv_bcast)

        wqt = singles.tile([P, NDQ, H], F32R, name="wqt")
        with tc.tile_pool(name="wqn", bufs=1) as wn_pool, \
             tc.tile_pool(name="wqp", bufs=2, space="PSUM") as wps:
            _transpose_weight(tc, wn_pool, wps, wq, wqt, NDQ, ident, "q")

        for b in range(BPC):
            with ExitStack() as bctx:
                ktv = bctx.enter_context(tc.tile_pool(name=f"ktv{b}", bufs=1))
                kt = ktv.tile([P, NH, KVLEN], F32R, name=f"kt{b}", tag="kt")
                vts = [ktv.tile([P, H], BF16, name=f"v{b}_{j}", tag=f"v{j}")
                       for j in range(NKT)]
                with tc.tile_pool(name=f"wt{b}", bufs=1) as wt_pool:
                    wkt = wt_pool.tile([P, NDK, H], F32R, name=f"wkt{b}", tag="wkt")
                    wvt = wt_pool.tile([P, NDK, H], F32R, name=f"wvt{b}", tag="wvt")
                    with tc.tile_pool(name=f"wn{b}", bufs=1) as wn_pool, \
                         tc.tile_pool(name=f"wp{b}", bufs=2, space="PSUM") as wps:
                        _transpose_weight(tc, wn_pool, wps, wk, wkt, NDK, ident, f"k{b}")
                        _transpose_weight(tc, wn_pool, wps, wv, wvt, NDK, ident, f"v{b}")
                    self_kv(tc, b, kvm, kt, vts, wkt, wvt, bks, bvb, ident)
                self_attn(tc, b, xq, out, kt, vts, wqt, bqs, ident, identb)


def self_kv(tc, b, kvm, kt, vts, wkt, wvt, bks, bvb, ident):
    """Project kv -> K^T (fp32 SBUF) and V (bf16 SBUF)."""
    nc = tc.nc
    with tc.tile_pool(name=f"kvn{b}", bufs=6) as kvn, \
         tc.tile_pool(name=f"kvt{b}", bufs=1) as kvtp, \
         tc.tile_pool(name=f"tp{b}", bufs=2, space="PSUM") as tps, \
         tc.tile_pool(name=f"kp{b}", bufs=2, space="PSUM") as kps, \
         tc.tile_pool(name=f"vp{b}", bufs=2, space="PSUM") as vps:
        for kb in range(KB):
            knats = []
            for j in range(4):
                kn = kvn.tile([P, DKV], F32, name=f"kvn{b}_{kb}_{j}", tag="kvn")
                nc.sync.dma_start(
                    out=kn, in_=kvm[b, kb * BLK + j * P:kb * BLK + (j + 1) * P, :])
                knats.append(kn)
            kvtb = kvtp.tile([P, NDK, BLK], F32R, name=f"kvtb{b}_{kb}", tag="kvtb")
            for d in range(NDK):
                ps = tps.tile([P, BLK], F32, name=f"tp{b}_{kb}_{d}", tag="tp")
                for j in range(4):
                    nc.tensor.transpose(ps[:, j * P:(j + 1) * P],
                                        knats[j][:, d * P:(d + 1) * P], ident)
                nc.vector.tensor_copy(out=kvtb[:, d, :], in_=ps)
            for h in range(NH):
                ps = kps.tile([P, BLK], F32, name=f"kp{b}_{kb}_{h}", tag="kp")
                for d in range(NDK):
                    nc.tensor.matmul(ps, wkt[:, d, h * P:(h + 1) * P],
                                     kvtb[:, d, :],
                                     start=(d == 0), stop=(d == NDK - 1))
                nc.scalar.activation(out=kt[:, h, kb * BLK:(kb + 1) * BLK], in_=ps,
                                     func=AF.Identity, bias=bks[:, h:h + 1], scale=1.0)
            for j in range(4):
                ki = kb * 4 + j
                ps = vps.tile([P, H], F32, name=f"vp{b}_{ki}", tag="vp")
                for d in range(NDK):
                    nc.tensor.matmul(ps[:, 0:BLK],
                                     kvtb[:, d, j * P:(j + 1) * P],
                                     wvt[:, d, 0:BLK],
                                     start=(d == 0), stop=(d == NDK - 1))
                for d in range(NDK):
                    nc.tensor.matmul(ps[:, BLK:H],
                                     kvtb[:, d, j * P:(j + 1) * P],
                                     wvt[:, d, BLK:H],
                                     start=(d == 0), stop=(d == NDK - 1))
                nc.vector.tensor_add(out=vts[ki], in0=ps, in1=bvb)


def self_attn(tc, b, xq, out, kt, vts, wqt, bqs, ident, identb):
    nc = tc.nc
    with tc.tile_pool(name=f"xqn{b}", bufs=6) as xqn, \
         tc.tile_pool(name=f"xqt{b}", bufs=1) as xqtp, \
         tc.tile_pool(name=f"qt{b}", bufs=1) as qtp, \
         tc.tile_pool(name=f"sp{b}", bufs=2, space="PSUM") as sps, \
         tc.tile_pool(name=f"t2{b}", bufs=2, space="PSUM") as tp2, \
         tc.tile_pool(name=f"op{b}", bufs=1, space="PSUM") as ops, \
         tc.tile_pool(name=f"pp{b}", bufs=2) as ppool, \
         tc.tile_pool(name=f"pt{b}", bufs=2) as ptpool, \
         tc.tile_pool(name=f"ot{b}", bufs=3) as otpool, \
         tc.tile_pool(name=f"sm{b}", bufs=3) as small:
        for qb in range(QB):
            qnats = []
            for j in range(4):
                qn = xqn.tile([P, DQ], F32, name=f"xqn{b}_{qb}_{j}", tag="xqn")
                nc.sync.dma_start(
                    out=qn, in_=xq[b, qb * BLK + j * P:qb * BLK + (j + 1) * P, :])
                qnats.append(qn)
            xqtb = xqtp.tile([P, NDQ, BLK], F32R, name=f"xqtb{b}_{qb}", tag="xqtb")
            for d in range(NDQ):
                ps = tp2.tile([P, BLK], F32, name=f"xp{b}_{qb}_{d}", tag="t2")
                for j in range(4):
                    nc.tensor.transpose(ps[:, j * P:(j + 1) * P],
                                        qnats[j][:, d * P:(d + 1) * P], ident)
                nc.vector.tensor_copy(out=xqtb[:, d, :], in_=ps)
            qtb = qtp.tile([P, NH, BLK], F32R, name=f"qtb{b}_{qb}", tag="qtb")
            for h in range(NH):
                ps = ops.tile([P, BLK], F32, name=f"qp{b}_{qb}_{h}", tag="op")
                for d in range(NDQ):
                    nc.tensor.matmul(ps, wqt[:, d, h * P:(h + 1) * P],
                                     xqtb[:, d, :],
                                     start=(d == 0), stop=(d == NDQ - 1))
                nc.scalar.activation(out=qtb[:, h, :], in_=ps, func=AF.Identity,
                                     bias=bqs[:, h:h + 1], scale=1.0)
            for qi in range(4):
                shalves = []
                for half in range(2):
                    st = sps.tile([P, 1024], F32, name=f"s{b}_{qb}_{qi}_{half}",
                                  tag="s")
                    for cc in range(2):
                        for h in range(NH):
                            nc.tensor.matmul(
                                st[:, cc * BLK:(cc + 1) * BLK],
                                qtb[:, h, qi * P:(qi + 1) * P],
                                kt[:, h, half * 1024 + cc * BLK:
                                       half * 1024 + (cc + 1) * BLK],
                                start=(h == 0), stop=(h == NH - 1))
                    shalves.append(st)
                # softmax: shift by max of the first half (shift-invariant;
                # keeps Exp inputs in a sane range), fused row-sums via ACT
                mx = small.tile([P, 1], F32, name=f"mx{b}_{qb}_{qi}", tag="mx")
                nc.vector.reduce_max(mx, shalves[0], axis=AX_X, negate=True)
                pa = ppool.tile([P, KVLEN], BF16, name=f"pa{b}_{qb}_{qi}", tag="pa")
                sm0 = small.tile([P, 1], F32, name=f"sm0_{b}_{qb}_{qi}", tag="sm0")
                sm1 = small.tile([P, 1], F32, name=f"sm1_{b}_{qb}_{qi}", tag="sm1")
                nc.scalar.activation(out=pa[:, 0:1024], in_=shalves[0], func=AF.Exp,
                                     bias=mx, scale=1.0, accum_out=sm0)
                nc.scalar.activation(out=pa[:, 1024:2048], in_=shalves[1],
                                     func=AF.Exp, bias=mx, scale=1.0, accum_out=sm1)
                rcp = small.tile([P, 1], F32, name=f"rcp{b}_{qb}_{qi}", tag="rcp")
                nc.vector.tensor_add(out=rcp, in0=sm0, in1=sm1)
                nc.vector.reciprocal(rcp, rcp)
                ptb = ptpool.tile([P, NKT, P], BF16, name=f"ptb{b}_{qb}_{qi}",
                                  tag="ptb")
                for g in range(4):
                    ps = tp2.tile([P, BLK], BF16, name=f"pt{b}_{qb}_{qi}_{g}",
                                  tag="t2")
                    for j in range(4):
                        cjk = (g * 4 + j) * P
                        nc.tensor.transpose(ps[:, j * P:(j + 1) * P],
                                            pa[:, cjk:cjk + P], identb)
                    nc.vector.tensor_copy(out=ptb[:, g * 4:(g + 1) * 4, :], in_=ps)
                po = ops.tile([P, H], F32, name=f"po{b}_{qb}_{qi}", tag="op")
                for j in range(NKT):
                    nc.tensor.matmul(po[:, 0:BLK], ptb[:, j, :], vts[j][:, 0:BLK],
                                     start=(j == 0), stop=(j == NKT - 1))
                for j in range(NKT):
                    nc.tensor.matmul(po[:, BLK:H], ptb[:, j, :], vts[j][:, BLK:H],
                                     start=(j == 0), stop=(j == NKT - 1))
                ot = otpool.tile([P, H], F32, name=f"ot{b}_{qb}_{qi}", tag="ot")
                nc.scalar.activation(out=ot, in_=po, func=AF.Copy, bias=0.0,
                                     scale=rcp)
                nc.sync.dma_start(
                    out=out[b, qb * BLK + qi * P:qb * BLK + (qi + 1) * P, :], in_=ot)


def build_program():
    nc = bacc.Bacc("TRN2", target_bir_lowering=False, debug=False,
                   enable_asserts=False, num_devices=NCORES)
    xq = nc.dram_tensor("xq", [BPC, QLEN, DQ], F32, kind="ExternalInput").ap()
    kvm = nc.dram_tensor("kvm", [BPC, KVLEN, DKV], F32, kind="ExternalInput").ap()
    wq = nc.dram_tensor("wq", [H, DQ], F32, kind="ExternalInput").ap()
    bq = nc.dram_tensor("bq", [H], F32, kind="ExternalInput").ap()
    wk = nc.dram_tensor("wk", [H, DKV], F32, kind="ExternalInput").ap()
    bk = nc.dram_tensor("bk", [H], F32, kind="ExternalInput").ap()
    wv = nc.dram_tensor("wv", [H, DKV], F32, kind="ExternalInput").ap()
    bv = nc.dram_tensor("bv", [H], F32, kind="ExternalInput").ap()
    out = nc.dram_tensor("out", [BPC, QLEN, H], F32, kind="ExternalOutput").ap()
    with tile.TileContext(nc) as tc:
        _emit(tc, xq, kvm, wq, bq, wk, bk, wv, bv, out)
    nc.compile()
    return nc


def make_in_maps(query_modality, kv_modality, Wq, bq, Wk, bk, Wv, bv):
    in_maps = []
    for c in range(NCORES):
        sl = slice(c * BPC, (c + 1) * BPC)
        in_maps.append({
            "xq": np.ascontiguousarray(query_modality[sl], dtype=np.float32),
            "kvm": np.ascontiguousarray(kv_modality[sl], dtype=np.float32),
            "wq": np.asarray(Wq, dtype=np.float32),
            "bq": np.asarray(bq, dtype=np.float32),
            "wk": np.asarray(Wk, dtype=np.float32),
            "bk": np.asarray(bk, dtype=np.float32),
            "wv": np.asarray(Wv, dtype=np.float32),
            "bv": np.asarray(bv, dtype=np.float32),
        })
    return in_maps


def kernel(query_modality, kv_modality, Wq, bq, Wk, bk, Wv, bv, **run_kwargs):
    nc = build_program()
    in_maps = make_in_maps(query_modality, kv_modality, Wq, bq, Wk, bk, Wv, bv)
    res = run_bass_kernel_spmd(nc, in_maps, core_ids=list(range(NCORES)),
                               **run_kwargs)
    out = np.concatenate([res.results[c]["out"] for c in range(NCORES)], axis=0)
    kernel.last_results = res
    return out


# revision 15
# speedup vs baseline: 1.3244x; 1.3244x over previous
"""Trainium2 Bass kernel for CrossModalAttention.

Full (unsharded) inputs in, full output out. Internally: data-parallel over
batch across 8 NeuronCores (B=16 -> 2 batches per core), one SPMD Bass/Tile
program per core, executed via run_bass_kernel_spmd.

Per-core algorithm (per batch):
  1. PE-transpose Wk/Wv (and Wq once) into [d, h] layout in SBUF.
  2. KV phase: stream kv in 512-row blocks; PE-transpose to kv^T; project to
     K^T [768, 2048] (fp32, +bias via ACT) and V [2048, 768] (bf16, +bias via
     DVE), both SBUF-resident.
  3. Attention phase: stream xq in 512-row blocks; PE-transpose; project to
     Q^T; per 128-row q-tile: S = Q^T.T @ K^T in two 1024-wide PSUM halves
     (float32r matmuls), softmax via ACT Exp with fused row-sum (max offset
     taken from the first half only - safe, softmax is shift-invariant),
     P stored bf16, PE-transposed to P^T, then O = P^T.T @ V accumulated over
     16 k-chunks, scaled by 1/sum on the way out.

Matmul dtype: float32r (TF32-like, full PE rate at moving-dim >= 256) for the
projections and scores; bf16 for P/V in the second attention matmul.
"""

import numpy as np
from contextlib import ExitStack

import concourse.bass as bass
import concourse.mybir as mybir
import concourse.tile as tile
from concourse import bacc
from concourse.bass_utils import run_bass_kernel_spmd
from concourse.masks import make_identity

F32 = mybir.dt.float32
F32R = mybir.dt.float32r
BF16 = mybir.dt.bfloat16
AX_X = mybir.AxisListType.X
AF = mybir.ActivationFunctionType

B, QLEN, KVLEN = 16, 2048, 2048
DQ, DKV, H = 768, 1024, 768
NCORES = 8
BPC = B // NCORES  # batches per core
P = 128
NH = H // P    # 6 h-chunks
NDQ = DQ // P  # 6 d-chunks (query dim)
NDK = DKV // P # 8 d-chunks (kv dim)
BLK = 512
KB = KVLEN // BLK  # 4 kv blocks
QB = QLEN // BLK   # 4 q blocks
NKT = KVLEN // P   # 16 kv tiles of 128


def _transpose_weight(tc, nat_pool, psum_pool, w_ap, wt_tile, nd, ident, nm):
    """w_ap: DRAM [H, nd*128] (torch Linear weight layout [out, in]).
    wt_tile: SBUF f32r [128, nd, H] holding W^T, i.e. wt[p, d, h] = W[h, d*128+p]."""
    nc = tc.nc
    nats = []
    for h in range(NH):
        wn = nat_pool.tile([P, nd * P], F32, name=f"wn_{nm}_{h}", tag=f"wn{h}")
        nc.sync.dma_start(out=wn, in_=w_ap[h * P:(h + 1) * P, :])
        nats.append(wn)
    for d in range(nd):
        ps = psum_pool.tile([P, H], F32, name=f"wtp_{nm}_{d}", tag="wtp")
        for h in range(NH):
            nc.tensor.transpose(ps[:, h * P:(h + 1) * P],
                                nats[h][:, d * P:(d + 1) * P], ident)
        nc.vector.tensor_copy(out=wt_tile[:, d, :], in_=ps)


def _emit(tc, xq, kvm, wq, bq, wk, bk, wv, bv, out):
    nc = tc.nc
    with ExitStack() as ctx:
        singles = ctx.enter_context(tc.tile_pool(name="singles", bufs=1))
        ident = singles.tile([P, P], F32, name="ident")
        make_identity(nc, ident)
        identb = singles.tile([P, P], BF16, name="identb")
        make_identity(nc, identb)
        # biases: bq/bk as [128, 6] (per-partition scalars per h-chunk),
        # bv broadcast to all partitions (added along the free dim of V)
        bqs = singles.tile([P, NH], F32, name="bqs")
        nc.gpsimd.dma_start(out=bqs, in_=bq.rearrange("(t p) -> p t", p=P))
        bks = singles.tile([P, NH], F32, name="bks")
        nc.gpsimd.dma_start(out=bks, in_=bk.rearrange("(t p) -> p t", p=P))
        bvb = singles.tile([P, H], F32, name="bvb")
        bv_bcast = bass.AP(tensor=bv.tensor, offset=bv.offset,
                           ap=[[0, P]] + list(bv.ap))
        nc.gpsimd.dma_start(out=bvb, in_=bv_bcast)

        wqt = singles.tile([P, NDQ, H], F32R, name="wqt")
        with tc.tile_pool(name="wqn", bufs=1) as wn_pool, \
             tc.tile_pool(name="wqp", bufs=2, space="PSUM") as wps:
            _transpose_weight(tc, wn_pool, wps, wq, wqt, NDQ, ident, "q")

        for b in range(BPC):
            with ExitStack() as bctx:
                ktv = bctx.enter_context(tc.tile_pool(name=f"ktv{b}", bufs=1))
                kt = ktv.tile([P, NH, KVLEN], F32R, name=f"kt{b}", tag="kt")
                vts = [ktv.tile([P, H], BF16, name=f"v{b}_{j}", tag=f"v{j}")
                       for j in range(NKT)]
                with tc.tile_pool(name=f"wt{b}", bufs=1) as wt_pool:
                    wkt = wt_pool.tile([P, NDK, H], F32R, name=f"wkt{b}", tag="wkt")
                    wvt = wt_pool.tile([P, NDK, H], F32R, name=f"wvt{b}", tag="wvt")
                    with tc.tile_pool(name=f"wn{b}", bufs=1) as wn_pool, \
                         tc.tile_pool(name=f"wp{b}", bufs=2, space="PSUM") as wps:
                        _transpose_weight(tc, wn_pool, wps, wk, wkt, NDK, ident, f"k{b}")
                        _transpose_weight(tc, wn_pool, wps, wv, wvt, NDK, ident, f"v{b}")
                    self_kv(tc, b, kvm, kt, vts, wkt, wvt, bks, bvb, ident)
                self_attn(tc, b, xq, out, kt, vts, wqt, bqs, ident, identb)


def self_kv(tc, b, kvm, kt, vts, wkt, wvt, bks, bvb, ident):
    """Project kv -> K^T (fp32 SBUF) and V (bf16 SBUF)."""
    nc = tc.nc
    with tc.tile_pool(name=f"kvn{b}", bufs=4) as kvn, \
         tc.tile_pool(name=f"kvt{b}", bufs=2) as kvtp, \
         tc.tile_pool(name=f"tp{b}", bufs=2, space="PSUM") as tps, \
         tc.tile_pool(name=f"kp{b}", bufs=2, space="PSUM") as kps, \
         tc.tile_pool(name=f"vp{b}", bufs=2, space="PSUM") as vps:
        for kb in range(KB):
            knats = []
            for j in range(4):
                kn = kvn.tile([P, DKV], F32, name=f"kvn{b}_{kb}_{j}", tag="kvn")
                nc.sync.dma_start(
                    out=kn, in_=kvm[b, kb * BLK + j * P:kb * BLK + (j + 1) * P, :])
                knats.append(kn)
            kvtb = kvtp.tile([P, NDK, BLK], F32R, name=f"kvtb{b}_{kb}", tag="kvtb")
            for d in range(NDK):
                ps = tps.tile([P, BLK], F32, name=f"tp{b}_{kb}_{d}", tag="tp")
                for j in range(4):
                    nc.tensor.transpose(ps[:, j * P:(j + 1) * P],
                                        knats[j][:, d * P:(d + 1) * P], ident)
                nc.vector.tensor_copy(out=kvtb[:, d, :], in_=ps)
            for h in range(NH):
                ps = kps.tile([P, BLK], F32, name=f"kp{b}_{kb}_{h}", tag="kp")
                for d in range(NDK):
                    nc.tensor.matmul(ps, wkt[:, d, h * P:(h + 1) * P],
                                     kvtb[:, d, :],
                                     start=(d == 0), stop=(d == NDK - 1))
                nc.scalar.activation(out=kt[:, h, kb * BLK:(kb + 1) * BLK], in_=ps,
                                     func=AF.Identity, bias=bks[:, h:h + 1], scale=1.0)
            for j in range(4):
                ki = kb * 4 + j
                ps = vps.tile([P, H], F32, name=f"vp{b}_{ki}", tag="vp")
                for d in range(NDK):
                    nc.tensor.matmul(ps[:, 0:BLK],
                                     kvtb[:, d, j * P:(j + 1) * P],
                                     wvt[:, d, 0:BLK],
                                     start=(d == 0), stop=(d == NDK - 1))
                for d in range(NDK):
                    nc.tensor.matmul(ps[:, BLK:H],
                                     kvtb[:, d, j * P:(j + 1) * P],
                                     wvt[:, d, BLK:H],
                                     start=(d == 0), stop=(d == NDK - 1))
                nc.vector.tensor_add(out=vts[ki], in0=ps, in1=bvb)


def self_attn(tc, b, xq, out, kt, vts, wqt, bqs, ident, identb):
    nc = tc.nc
    with tc.tile_pool(name=f"xqn{b}", bufs=6) as xqn, \
         tc.tile_pool(name=f"xqt{b}", bufs=2) as xqtp, \
         tc.tile_pool(name=f"qt{b}", bufs=2) as qtp, \
         tc.tile_pool(name=f"sp{b}", bufs=2, space="PSUM") as sps, \
         tc.tile_pool(name=f"t2{b}", bufs=2, space="PSUM") as tp2, \
         tc.tile_pool(name=f"op{b}", bufs=1, space="PSUM") as ops, \
         tc.tile_pool(name=f"pp{b}", bufs=2) as ppool, \
         tc.tile_pool(name=f"pt{b}", bufs=2) as ptpool, \
         tc.tile_pool(name=f"ot{b}", bufs=3) as otpool, \
         tc.tile_pool(name=f"sm{b}", bufs=3) as small:
        for qb in range(QB):
            qnats = []
            for j in range(4):
                qn = xqn.tile([P, DQ], F32, name=f"xqn{b}_{qb}_{j}", tag="xqn")
                nc.sync.dma_start(
                    out=qn, in_=xq[b, qb * BLK + j * P:qb * BLK + (j + 1) * P, :])
                qnats.append(qn)
            xqtb = xqtp.tile([P, NDQ, BLK], F32R, name=f"xqtb{b}_{qb}", tag="xqtb")
            for d in range(NDQ):
                ps = tp2.tile([P, BLK], F32, name=f"xp{b}_{qb}_{d}", tag="t2")
                for j in range(4):
                    nc.tensor.transpose(ps[:, j * P:(j + 1) * P],
                                        qnats[j][:, d * P:(d + 1) * P], ident)
                nc.vector.tensor_copy(out=xqtb[:, d, :], in_=ps)
            qtb = qtp.tile([P, NH, BLK], F32R, name=f"qtb{b}_{qb}", tag="qtb")
            for h in range(NH):
                ps = ops.tile([P, BLK], F32, name=f"qp{b}_{qb}_{h}", tag="op")
                for d in range(NDQ):
                    nc.tensor.matmul(ps, wqt[:, d, h * P:(h + 1) * P],
                                     xqtb[:, d, :],
                                     start=(d == 0), stop=(d == NDQ - 1))
                nc.scalar.activation(out=qtb[:, h, :], in_=ps, func=AF.Identity,
                                     bias=bqs[:, h:h + 1], scale=1.0)
            for qi in range(4):
                shalves = []
                for half in range(2):
                    st = sps.tile([P, 1024], F32, name=f"s{b}_{qb}_{qi}_{half}",
                                  tag="s")
                    for cc in range(2):
                        for h in range(NH):
                            nc.tensor.matmul(
                                st[:, cc * BLK:(cc + 1) * BLK],
                                qtb[:, h, qi * P:(qi + 1) * P],
                                kt[:, h, half * 1024 + cc * BLK:
                                       half * 1024 + (cc + 1) * BLK],
                                start=(h == 0), stop=(h == NH - 1))
                    shalves.append(st)
                # softmax: shift by max of the first half (shift-invariant;
                # keeps Exp inputs in a sane range), fused row-sums via ACT
                mx = small.tile([P, 1], F32, name=f"mx{b}_{qb}_{qi}", tag="mx")
                nc.vector.reduce_max(mx, shalves[0], axis=AX_X, negate=True)
                pa = ppool.tile([P, KVLEN], BF16, name=f"pa{b}_{qb}_{qi}", tag="pa")
                sm0 = small.tile([P, 1], F32, name=f"sm0_{b}_{qb}_{qi}", tag="sm0")
                sm1 = small.tile([P, 1], F32, name=f"sm1_{b}_{qb}_{qi}", tag="sm1")
                nc.scalar.activation(out=pa[:, 0:1024], in_=shalves[0], func=AF.Exp,
                                     bias=mx, scale=1.0, accum_out=sm0)
                nc.scalar.activation(out=pa[:, 1024:2048], in_=shalves[1],
                                     func=AF.Exp, bias=mx, scale=1.0, accum_out=sm1)
                rcp = small.tile([P, 1], F32, name=f"rcp{b}_{qb}_{qi}", tag="rcp")
                nc.vector.tensor_add(out=rcp, in0=sm0, in1=sm1)
                nc.vector.reciprocal(rcp, rcp)
                ptb = ptpool.tile([P, NKT, P], BF16, name=f"ptb{b}_{qb}_{qi}",
                                  tag="ptb")
                for g in range(4):
                    ps = tp2.tile([P, BLK], BF16, name=f"pt{b}_{qb}_{qi}_{g}",
                                  tag="t2")
                    for j in range(4):
                        cjk = (g * 4 + j) * P
                        nc.tensor.transpose(ps[:, j * P:(j + 1) * P],
                                            pa[:, cjk:cjk + P], identb)
                    nc.vector.tensor_copy(out=ptb[:, g * 4:(g + 1) * 4, :], in_=ps)
                po = ops.tile([P, H], F32, name=f"po{b}_{qb}_{qi}", tag="op")
                for j in range(NKT):
                    nc.tensor.matmul(po[:, 0:BLK], ptb[:, j, :], vts[j][:, 0:BLK],
                                     start=(j == 0), stop=(j == NKT - 1))
                for j in range(NKT):
                    nc.tensor.matmul(po[:, BLK:H], ptb[:, j, :], vts[j][:, BLK:H],
                                     start=(j == 0), stop=(j == NKT - 1))
                ot = otpool.tile([P, H], F32, name=f"ot{b}_{qb}_{qi}", tag="ot")
                nc.scalar.activation(out=ot, in_=po, func=AF.Copy, bias=0.0,
                                     scale=rcp)
                nc.sync.dma_start(
                    out=out[b, qb * BLK + qi * P:qb * BLK + (qi + 1) * P, :], in_=ot)


def build_program():
    nc = bacc.Bacc("TRN2", target_bir_lowering=False, debug=False,
                   enable_asserts=False, num_devices=NCORES)
    xq = nc.dram_tensor("xq", [BPC, QLEN, DQ], F32, kind="ExternalInput").ap()
    kvm = nc.dram_tensor("kvm", [BPC, KVLEN, DKV], F32, kind="ExternalInput").ap()
    wq = nc.dram_tensor("wq", [H, DQ], F32, kind="ExternalInput").ap()
    bq = nc.dram_tensor("bq", [H], F32, kind="ExternalInput").ap()
    wk = nc.dram_tensor("wk", [H, DKV], F32, kind="ExternalInput").ap()
    bk = nc.dram_tensor("bk", [H], F32, kind="ExternalInput").ap()
    wv = nc.dram_tensor("wv", [H, DKV], F32, kind="ExternalInput").ap()
    bv = nc.dram_tensor("bv", [H], F32, kind="ExternalInput").ap()
    out = nc.dram_tensor("out", [BPC, QLEN, H], F32, kind="ExternalOutput").ap()
    with tile.TileContext(nc) as tc:
        _emit(tc, xq, kvm, wq, bq, wk, bk, wv, bv, out)
    nc.compile()
    return nc


def make_in_maps(query_modality, kv_modality, Wq, bq, Wk, bk, Wv, bv):
    in_maps = []
    for c in range(NCORES):
        sl = slice(c * BPC, (c + 1) * BPC)
        in_maps.append({
            "xq": np.ascontiguousarray(query_modality[sl], dtype=np.float32),
            "kvm": np.ascontiguousarray(kv_modality[sl], dtype=np.float32),
            "wq": np.asarray(Wq, dtype=np.float32),
            "bq": np.asarray(bq, dtype=np.float32),
            "wk": np.asarray(Wk, dtype=np.float32),
            "bk": np.asarray(bk, dtype=np.float32),
            "wv": np.asarray(Wv, dtype=np.float32),
            "bv": np.asarray(bv, dtype=np.float32),
        })
    return in_maps


def kernel(query_modality, kv_modality, Wq, bq, Wk, bk, Wv, bv, **run_kwargs):
    import os
    # NTFF tracing under axon needs antenv.axon_hooks, which this container
    # lacks; make sure an ambient BASS_TRACE can't crash the run.
    os.environ.setdefault("BASS_NEVER_TRACE", "1")
    nc = build_program()
    in_maps = make_in_maps(query_modality, kv_modality, Wq, bq, Wk, bk, Wv, bv)
    res = run_bass_kernel_spmd(nc, in_maps, core_ids=list(range(NCORES)),
                               **run_kwargs)
    out = np.concatenate([res.results[c]["out"] for c in range(NCORES)], axis=0)
    kernel.last_results = res
    return out
